# revision 2
# baseline (speedup 1.0000x reference)
"""DepGcn forward kernel for Trainium2 (Bass/Tile), 8-core data-parallel.

Math (per batch b, handled by one NeuronCore):
    t[i,e] = sum_j adj[i,j] * (hidden[j,e] + dep_embed[j,i,e])
    out[i,d] = t[i,:] @ W[:,d] + bias[d]

The reference materializes fusion = (hidden+dep) @ W ([N,N,D] sized); we
instead reduce over j first, which makes the kernel purely HBM-bound on
streaming dep_embed (33.55 MB/core/rep).

Measured DMA facts on this fleet (pure-stream microbenches, slope-timed):
  - The dep stream alone runs at ~94.1 us/rep (~356 GB/s/core), right at
    the HBM-per-NC limit (716 GB/s/stack shared by 2 NCs). DMA size
    (2/4/8 MB) and issue path (SWDGE vs HWDGE, cast vs plain) make no
    difference; multi-queue issue is WORSE (engines round-robin between
    rings at packet granularity: 2 rings +4 us, 3 rings +9 us on
    continuous streams; a brief burst of side DMAs costs only ~0.5 us).
  - tc.For_i costs ~15-19 us per trip (all-engine barrier + semaphore
    reset drains the DMA pipeline at the back edge); test.py dilutes it
    with LOOP_UNROLL=64 (~0.3 us/rep residue).
  - A plain fp32 DMA + bitcast-to-f32r view for the PE is rejected by
    the BIR verifier (checkMatmultFP32r: producer must round), so the
    dep stream uses SWDGE fp32->fp32r casting DMAs (the cast is free).

Implementation notes:
  - dep_embed[b] is streamed with j on SBUF partitions:
    tile[j, (i_local, e)] <- dep[jc*128+j, i0:i0+ICHUNK, :]  (2 MB per
    DMA, 16 KB contiguous per partition) on the gpsimd SWDGE queue.
  - The weighted j-reduction runs on the TensorEngine as diagonal-block
    matmuls in float32r (full-rate fp32 path for moving dim >= 256):
    psum[4,512] = adjT[:, i0:i0+4].T @ dep_tile[:, 512-slice]; only the
    4 diagonal [1,128] strips (row m, cols 128m..128m+128) are useful.
    PSUM accumulates the two 128-wide j-chunks (start/stop flags).
  - Whole [4,512] psum tiles are copied to SBUF on the VectorEngine;
    the diagonal strips are then scattered into the [128,.] accumulator
    with small SBUF->SBUF DMAs on the scalar HWDGE ring (DMA has no
    partition-alignment limits; compute engines can only address
    partition offsets 0/32/64/96).
  - t1 = adj @ hidden is parked in a persistent PSUM bank: the per-rep
    DVE adds then read (SBUF t2h + PSUM t1) via DVE dedicated ports
    only. A (SBUF+SBUF) fp32 DVE op grabs the shared DVE/GpSimd SBUF
    port pair, which locks GpSimd out and can starve SWDGE descriptor
    generation for the dep stream.
  - The final projection (@W + bias) are small PE matmuls; bias is
    added via a K=1 matmul with a ones row.
"""

import numpy as np

B, N, D = 8, 256, 128
NCORES = 8
ICHUNK = 32   # i's per dep tile (2 MB DMAs)
IGROUP = 4    # i's per diagonal-block matmul (rhs N = IGROUP*D = 512)
# "cast" streams dep via SWDGE fp32->fp32r casting DMAs. f32r is a rounded
# (tf32-like) format: the BIR verifier requires fp32r-matmul inputs to be
# rounded by the producer, so a plain fp32 DMA + bitcast view is rejected.
# Measured (calib3): all DMA paths (sync/alt HWDGE, SWDGE, SWDGE+cast) hit
# the same ~99 us/rep = ~340 GB/s/core HBM floor, so the cast is free.
DEP_DMA = "cast"   # "alt" | "sync" | "gps" | "cast"

_CACHE = {}


def _build_bass(reps=1, loop_unroll=None):
    """Build the Bass program. reps>1 repeats the whole streaming body
    serially inside one NEFF. loop_unroll=k instead wraps k unrolled reps
    in a tc.For_i whose trip count is the runtime input "nreps" — used
    for steady-state timing with large rep counts (same executable for
    every timing point)."""
    import concourse.bass as bass
    import concourse.mybir as mybir
    import concourse.tile as tile
    from concourse import bacc
    from concourse.masks import make_identity

    f32 = mybir.dt.float32
    f32r = mybir.dt.float32r
    nc = bacc.Bacc("TRN2", target_bir_lowering=False, debug=False)

    hid_d = nc.dram_tensor("hidden", [N, D], f32, kind="ExternalInput").ap()
    adj_d = nc.dram_tensor("adj", [N, N], f32, kind="ExternalInput").ap()
    dep_d = nc.dram_tensor("dep", [N, N, D], f32, kind="ExternalInput").ap()
    w_d = nc.dram_tensor("weight", [D, D], f32, kind="ExternalInput").ap()
    b_d = nc.dram_tensor("bias", [1, D], f32, kind="ExternalInput").ap()
    nreps_d = None
    if loop_unroll is not None:
        nreps_d = nc.dram_tensor(
            "nreps", [1, 1], mybir.dt.int32, kind="ExternalInput"
        ).ap()
    out_d = nc.dram_tensor("out", [N, D], f32, kind="ExternalOutput").ap()

    # chunk schedule (global i-range per chunk); the final chunks of each
    # half are small so the exposed matmul/copy/scatter tail after the last
    # dep transfer is short.
    CHUNKS = [(i0, 32) for i0 in range(0, 256, 32)]
    assert sum(s for _, s in CHUNKS) == N

    dep_tile_dt = f32r if DEP_DMA == "cast" else f32

    with tile.TileContext(nc) as tc:
        with (
            tc.tile_pool(name="const", bufs=1) as cpool,
            tc.tile_pool(name="deps", bufs=8) as dpool,
            tc.tile_pool(name="accs", bufs=1) as apool,
            tc.tile_pool(name="psg", bufs=4, space="PSUM") as psg,
            tc.tile_pool(name="psm", bufs=2, space="PSUM") as psm,
            tc.tile_pool(name="pst1", bufs=1, space="PSUM") as pst1p,
        ):
            dep_r = dep_d.rearrange("(jc j) i e -> jc j (i e)", j=128)

            dep_dma_count = [0]

            def dep_engine():
                if DEP_DMA in ("gps", "cast"):
                    return nc.gpsimd
                if DEP_DMA == "sync":
                    return nc.sync
                dep_dma_count[0] += 1
                return nc.sync if dep_dma_count[0] % 2 else nc.scalar

            def load_chunk(i0, isz):
                tiles = []
                for jc in range(2):
                    t = dpool.tile([128, ICHUNK * D], dep_tile_dt, name="dep_t")
                    dep_engine().dma_start(
                        t[:, :isz * D], dep_r[jc, :, i0 * D:(i0 + isz) * D]
                    )
                    tiles.append(t)
                return tiles

            def mm_view(t):
                return t if DEP_DMA == "cast" else t.bitcast(f32r)

            # issue the first chunk's streaming DMAs before anything else so
            # the DMA engines are busy from t=0 (plain build only; the loop
            # build's body must be iteration-generic)
            pre_tiles = load_chunk(*CHUNKS[0]) if loop_unroll is None else None

            if nreps_d is not None:
                nreps_sb = cpool.tile([1, 1], mybir.dt.int32, name="nreps_sb")
                nc.gpsimd.dma_start(nreps_sb[:], nreps_d[:])
                n_iters = nc.values_load(nreps_sb[:], min_val=1, max_val=1 << 20,
                                         skip_runtime_bounds_check=True)

            # constants go on the scalar HWDGE ring: the dep stream owns the
            # gpsimd SWDGE queue, so these don't wait behind 2 MB transfers
            ident = cpool.tile([128, 128], f32, name="ident")
            make_identity(nc, ident[:])

            w_sb = cpool.tile([D, D], f32, name="w_sb")
            nc.scalar.dma_start(w_sb[:], w_d[:])
            bias_sb = cpool.tile([1, D], f32, name="bias_sb")
            nc.scalar.dma_start(bias_sb[:], b_d[:])
            ones_sb = cpool.tile([1, 128], f32, name="ones_sb")
            nc.gpsimd.memset(ones_sb[:], 1.0)

            # hidden[j,e] with j split into two 128-partition chunks
            hid_sb = cpool.tile([128, 2, D], f32, name="hid_sb")
            nc.scalar.dma_start(hid_sb[:], hid_d.rearrange("(jc j) e -> j jc e", j=128))
            # adj[i,j] with i split into two halves on partitions
            adj_sb = cpool.tile([128, 2, N], f32, name="adj_sb")
            nc.scalar.dma_start(adj_sb[:], adj_d.rearrange("(ih i) j -> i ih j", i=128))

            # adjT[jc][j, i] = adj[i, jc*128+j]  (PE transposes of 128x128 blocks)
            adjT = [cpool.tile([128, N], f32r, name=f"adjT{jc}")
                    for jc in range(2)]
            for jc in range(2):
                for ih in range(2):
                    ps = psm.tile([128, 128], f32, name="ps_tr", tag="psm")
                    nc.tensor.transpose(
                        ps[:], adj_sb[:, ih, jc * 128:(jc + 1) * 128], ident[:]
                    )
                    nc.vector.tensor_copy(adjT[jc][:, ih * 128:(ih + 1) * 128], ps[:])

            # Row permutation: within each 128-row half, perm row r = m*32 + G
            # holds natural i_in_half = G*4 + m (m-major), so each diagonal
            # strip scatter DMA writes a contiguous partition range. The final
            # output DMA un-permutes on the DRAM side.
            def perm_cols(ap2d, ih):
                return ap2d[:, ih * 128:(ih + 1) * 128].rearrange(
                    "j (G m) -> j m G", G=32, m=IGROUP
                )

            # adjT with columns permuted to (m, G) order, materialized so the
            # t1 matmul weights have a contiguous AP (walrus rejects multi-dim
            # weight APs)
            adjTp = [cpool.tile([128, N], f32, name=f"adjTp{jc}")
                     for jc in range(2)]
            for jc in range(2):
                for ih in range(2):
                    nc.vector.tensor_copy(
                        adjTp[jc][:, ih * 128:(ih + 1) * 128].rearrange(
                            "j (m G) -> j m G", m=IGROUP, G=32
                        ),
                        perm_cols(adjT[jc], ih),
                    )

            # term1[i,e] = sum_j adj[i,j] * hidden[j,e]   (rows in perm
            # order), parked in a persistent PSUM bank: the per-rep DVE add
            # then reads (SBUF t2h + PSUM t1) via DVE's dedicated ports only.
            # A (SBUF+SBUF) add would grab the shared DVE/GpSimd SBUF port
            # pair, locking GpSimd out and starving SWDGE descriptor
            # generation for the dep stream.
            t1ps = pst1p.tile([128, 2, D], f32, name="t1ps")
            for ih in range(2):
                for jc in range(2):
                    nc.tensor.matmul(
                        t1ps[:, ih, :],
                        adjTp[jc][:, ih * 128:(ih + 1) * 128],
                        hid_sb[:, jc, :],
                        start=(jc == 0),
                        stop=(jc == 1),
                    )

            def scatter_and_epilogue(ih, t2h, sb4h):
                # per quarter m: scatter strips -> t2 rows [32m,32m+32), then
                # add t1 and PE-transpose that quarter into psT[:, 32m:+32]
                acc = apool.tile([128, D], f32, name=f"acc{ih}")
                psT = psm.tile([128, 128], f32, name="ps_accT", tag="psm")
                for m in range(IGROUP):
                    src = sb4h[m:m + 1, :, m * D:(m + 1) * D]
                    q = slice(m * 32, (m + 1) * 32)
                    nc.scalar.dma_start(t2h[q, :], src)
                    nc.vector.tensor_add(acc[q, :], t2h[q, :], t1ps[q, ih, :])
                    nc.tensor.transpose(psT[:, q], acc[q, :],
                                        ident[q, q],
                                        tile_position=(m * 32, 0))
                accT = apool.tile([128, 128], f32, name=f"accT{ih}")
                nc.vector.tensor_copy(accT[:], psT[:])
                ps_out = psm.tile([128, D], f32, name="ps_out", tag="psm")
                nc.tensor.matmul(ps_out[:], accT[:], w_sb[:],
                                 start=True, stop=False)
                nc.tensor.matmul(
                    ps_out[:], ones_sb[:], bias_sb[:], start=False, stop=True
                )
                out_sb = apool.tile([128, D], f32, name=f"out{ih}")
                nc.vector.tensor_copy(out_sb[:], ps_out[:])
                # un-permute rows on the DRAM side: view position (m, G)
                # addresses out_d row G*4 + m
                dst = out_d[ih * 128:(ih + 1) * 128, :].rearrange(
                    "(G m) d -> m G d", G=32, m=IGROUP
                )
                nc.scalar.dma_start(dst, out_sb[:])

            # t2[i,e] = sum_j adj[i,j] * dep[j,i,e]   (diag-block matmuls)
            def one_rep(use_pre_tiles):
                t2h = [
                    apool.tile([128, D], f32, name=f"t2h{ih}") for ih in range(2)
                ]
                sb4h = None
                for ci, (i0, isz) in enumerate(CHUNKS):
                    if use_pre_tiles and ci == 0:
                        tiles = pre_tiles
                    else:
                        tiles = load_chunk(i0, isz)
                    if i0 % 128 == 0:
                        # one strip buffer per 128-row half
                        sb4h = apool.tile([IGROUP, 32, IGROUP * D], f32,
                                          name="sb4h", bufs=1)
                    gpc = isz // IGROUP
                    G0 = (i0 % 128) // IGROUP
                    for g in range(gpc):
                        ig = i0 + g * IGROUP
                        ps = psg.tile([IGROUP, IGROUP * D], f32, name="ps_g")
                        for jc in range(2):
                            nc.tensor.matmul(
                                ps[:],
                                adjT[jc][:, ig:ig + IGROUP],
                                mm_view(tiles[jc][:, g * IGROUP * D:(g + 1) * IGROUP * D]),
                                start=(jc == 0),
                                stop=(jc == 1),
                            )
                        nc.vector.tensor_copy(sb4h[:, G0 + g, :], ps[:])
                    if i0 + isz == 128 or i0 + isz == 256:
                        ih = i0 // 128
                        scatter_and_epilogue(ih, t2h[ih], sb4h)

            if loop_unroll is None:
                for _rep in range(reps):
                    one_rep(_rep == 0)
            else:
                # PE's per-iteration stream exceeds one 16 KiB IRAM block, so
                # hint the back-edge target to avoid a ~4 us I$ miss per trip
                # (DVE crosses a block too once unroll reaches 8)
                hints = (mybir.EngineType.PE,) if loop_unroll < 8 else (
                    mybir.EngineType.PE, mybir.EngineType.DVE)
                with tc.For_i(0, n_iters, hint_engines=hints) as _i:
                    for _u in range(loop_unroll):
                        one_rep(False)

    nc.compile()
    return nc


def _get_nc(reps=1, loop_unroll=None):
    key = ("nc", reps, loop_unroll)
    if key not in _CACHE:
        _CACHE[key] = _build_bass(reps, loop_unroll)
    return _CACHE[key]


def _get_runner(reps=1, loop_unroll=None):
    """Build (once) a sharded-jit callable running the bass NEFF on 8 cores.

    Mirrors concourse.bass2jax.run_bass_via_pjrt's multi-core branch, but
    exposes the jitted function + input ordering so callers can device_put
    inputs ahead of time and time pure device execution.
    """
    key = ("runner", reps, loop_unroll)
    if key in _CACHE:
        return _CACHE[key]

    import jax
    from jax.experimental.shard_map import shard_map
    from jax.sharding import Mesh, PartitionSpec

    import concourse.mybir as mybir
    from concourse import bass2jax

    nc = _get_nc(reps, loop_unroll)
    bass2jax.install_neuronx_cc_hook()

    partition_name = nc.partition_id_tensor.name if nc.partition_id_tensor else None
    in_names, out_names, out_avals, zero_outs = [], [], [], []
    for alloc in nc.m.functions[0].allocations:
        if not isinstance(alloc, mybir.MemoryLocationSet):
            continue
        name = alloc.memorylocations[0].name
        if alloc.kind == "ExternalInput":
            if name != partition_name:
                in_names.append(name)
        elif alloc.kind == "ExternalOutput":
            out_names.append(name)
            shape = tuple(alloc.tensor_shape)
            dtype = mybir.dt.np(alloc.dtype)
            out_avals.append(jax.core.ShapedArray(shape, dtype))
            zero_outs.append(np.zeros(shape, dtype))
    n_params = len(in_names)
    all_in_names = in_names + out_names
    if partition_name is not None:
        all_in_names = all_in_names + [partition_name]

    def _body(*args):
        operands = list(args)
        if partition_name is not None:
            operands.append(bass2jax.partition_id_tensor())
        outs = bass2jax._bass_exec_p.bind(
            *operands,
            out_avals=tuple(out_avals),
            in_names=tuple(all_in_names),
            out_names=tuple(out_names),
            lowering_input_output_aliases=(),
            sim_require_finite=True,
            sim_require_nnan=True,
            nc=nc,
        )
        return tuple(outs)

    devices = jax.devices()[:NCORES]
    mesh = Mesh(np.asarray(devices), ("core",))
    n_outs = len(out_names)
    sharded = jax.jit(
        shard_map(
            _body,
            mesh=mesh,
            in_specs=(PartitionSpec("core"),) * (n_params + n_outs),
            out_specs=(PartitionSpec("core"),) * n_outs,
            check_rep=False,
        ),
        keep_unused=True,
    )
    _CACHE[key] = (sharded, in_names, out_names, out_avals, zero_outs, mesh)
    return _CACHE[key]


def _concat_inputs(hidden, adj, dep_embed, weight, bias):
    """Per-core input dict -> concatenated global arrays in in_names order."""
    per_core = {
        "hidden": hidden,
        "adj": adj,
        "dep": dep_embed,
        "weight": np.broadcast_to(weight[None], (NCORES,) + weight.shape),
        "bias": np.broadcast_to(bias[None], (NCORES,) + bias.shape),
    }
    _, in_names, _, _, _, _ = _get_runner()
    return [
        np.ascontiguousarray(
            per_core[n].reshape(-1, *per_core[n].shape[2:])
        )
        for n in in_names
    ]


def run_spmd(hidden, adj, dep_embed, weight, bias_weight):
    """Run the kernel on all 8 cores; returns out [B,N,D]."""
    hidden = np.ascontiguousarray(np.asarray(hidden), dtype=np.float32)
    adj = np.ascontiguousarray(np.asarray(adj), dtype=np.float32)
    dep_embed = np.ascontiguousarray(np.asarray(dep_embed), dtype=np.float32)
    weight = np.ascontiguousarray(np.asarray(weight), dtype=np.float32)
    bias = np.ascontiguousarray(np.asarray(bias_weight), dtype=np.float32).reshape(
        1, D
    )

    sharded, in_names, out_names, out_avals, zero_outs, mesh = _get_runner()
    concat_in = _concat_inputs(hidden, adj, dep_embed, weight, bias)
    concat_zeros = [
        np.zeros((NCORES * z.shape[0], *z.shape[1:]), z.dtype) for z in zero_outs
    ]
    out_arrs = sharded(*concat_in, *concat_zeros)
    oi = out_names.index("out")
    out = np.asarray(out_arrs[oi]).reshape(NCORES, *out_avals[oi].shape)
    return out.astype(np.float32)


def kernel(hidden, adj, dep_embed, weight, bias_weight):
    return run_spmd(hidden, adj, dep_embed, weight, bias_weight)

